# revision 23
# baseline (speedup 1.0000x reference)
"""3x3 valid cross-correlation of a 4096x4096 fp32 image + scalar bias,
sharded row-wise across 8 TRN2 NeuronCores.

Memory-bound problem, so the kernel trades precision for HBM bandwidth
inside the harness's rel_err < 2e-2 budget: the image is converted to
bf16 on the host, the conv runs bf16 x bf16 -> fp32 PSUM on device, the
result is stored as bf16 and upcast to fp32 on the host. Total HBM
traffic per core drops from ~16.8 MB (fp32 in+out) to ~8.5 MB, and the
measured numeric error is ~4.5e-3.

Work split: the PE matmul stream is the critical path (moving-operand
cycles = passes x width x KW, independent of the panel height), so the
4094 output rows are split into 32 full 126-row panels (4 per core,
full width) plus one 62-row bottom strip that is sharded by COLUMN
across the cores (512 cols each). This gives every core 4 full-width
passes + one 1/8-width pass instead of 5 full-width passes, cutting PE
time by ~17%.

Per core:
  - Banded matmul per panel: for each kernel column dc, a stationary
    matrix B_dc[k, m] = w[k-m, dc] (k-m in 0..2) gives
    psum[m, n] += sum_k B_dc[k, m] * x[k, n+dc].
    The dc loop is OUTER over the column groups so the PE re-loads each
    stationary matrix 3x per panel instead of 24x; the 8 groups
    accumulate into the 8 PSUM banks (interleaved accumulation groups).
  - The strip (62 rows, K=64, one 512-col group) runs FIRST: its small
    load lands quickly and its matmuls double as the PE clock warmup.
  - Panel 0 is loaded in 2 column chunks so its first matmuls start
    after half a panel of DMA latency; panels 1-3 are single full-width
    loads (fully contiguous in HBM, 8 KB descriptors).
  - x loads ride the SP HWDGE ring; weights/bias/stores ride ACT.
  - Drains alternate ScalarE activation (bias fused) and VectorE
    tensor_scalar_add; both fuse the fp32 -> bf16 convert. Two stores
    per panel (one per 2048-col half) keep the store tail short.
"""

import ml_dtypes
import numpy as np

import concourse.bacc as bacc
import concourse.mybir as mybir
from concourse import tile
from concourse.bass_utils import run_bass_kernel_spmd

H, W = 4096, 4096
KH, KW = 3, 3
OH, OW = H - KH + 1, W - KW + 1  # 4094, 4094
NCORES = 8
PANEL_OUT = 126                  # output rows per full 128-input-row panel
N_PANELS = 4                     # full panels per core
ROWS_PER_CORE = N_PANELS * PANEL_OUT  # 504 full-width output rows per core
IN_ROWS = ROWS_PER_CORE + KH - 1      # 506 input rows per core
STRIP_ROWS = OH - NCORES * ROWS_PER_CORE  # 62 leftover rows (shared strip)
STRIP_IN = STRIP_ROWS + KH - 1            # 64
STRIP_COLS = 512                 # strip columns per core
STRIP_IN_COLS = 520              # loaded strip cols (512 + 2 halo, padded)
STRIP_R0 = NCORES * ROWS_PER_CORE         # 4032, first strip output row
COLS_PER_MM = 512                # one fp32 PSUM bank per 512-col group
CHUNK = 2048                     # columns per panel-0 chunk / store chunk
N_GROUPS = (OW + COLS_PER_MM - 1) // COLS_PER_MM  # 8 (last group is 510 wide)

_F32 = mybir.dt.float32
_BF16 = mybir.dt.bfloat16
_NP_BF16 = ml_dtypes.bfloat16

_PROGRAM_CACHE = None
last_results = None  # BassKernelResults of the most recent kernel() call


def _build_program():
    nc = bacc.Bacc(
        "TRN2", target_bir_lowering=False, debug=False, num_devices=NCORES
    )
    x = nc.dram_tensor("x", [IN_ROWS, W], _BF16, kind="ExternalInput")
    xs = nc.dram_tensor("xs", [STRIP_IN, STRIP_IN_COLS], _BF16, kind="ExternalInput")
    # Banded weights with the (bf16) bias packed as column 378 -- one
    # DMA dispatch, first on the SP ring, gates the first real matmul.
    wb = nc.dram_tensor("wb", [128, KW * PANEL_OUT + 2], _BF16, kind="ExternalInput")
    y = nc.dram_tensor("y", [ROWS_PER_CORE, OW], _BF16, kind="ExternalOutput")
    ys = nc.dram_tensor("ys", [STRIP_ROWS, STRIP_COLS], _BF16, kind="ExternalOutput")

    with tile.TileContext(nc) as tc:
        with (
            tc.tile_pool(name="const", bufs=1) as cpool,
            tc.tile_pool(name="xp", bufs=3) as xpool,
            tc.tile_pool(name="op", bufs=3) as opool,
            tc.tile_pool(name="pp", bufs=8, space="PSUM") as ppool,
        ):
            wt = cpool.tile([128, KW * PANEL_OUT + 2], _BF16)
            nc.sync.dma_start(wt[:], wb[:])
            # DVE wants an fp32 scalar operand: up-convert the packed
            # bf16 bias column once (runs in parallel with the warmup).
            bt = cpool.tile([128, 1], _F32)
            nc.vector.tensor_copy(bt[:], wt[:, KW * PANEL_OUT : KW * PANEL_OUT + 1])

            # PE warmup on a memset tile (no DMA dependency): the PE queue
            # opens ~6us in while the first x chunk lands ~10us in; these
            # throwaway matmuls bridge the gap and ramp the PE clock so
            # the real stream runs at full speed from its first matmul.
            wz = cpool.tile([128, 640], _BF16)
            nc.vector.memset(wz[:], 0.0)
            for i in range(14):
                nw = COLS_PER_MM if i < 6 else 128
                pswarm = ppool.tile(
                    [128, COLS_PER_MM], _F32, tag="ps", name="pswarm"
                )
                nc.tensor.matmul(
                    pswarm[:126, :nw],
                    wz[:, :126],
                    wz[:, 128 : 128 + nw],
                    start=True,
                    stop=True,
                    skip_group_check=True,
                )

            xst = cpool.tile([128, STRIP_IN_COLS], _BF16)

            def emit_strip():
                pstrip = ppool.tile(
                    [128, COLS_PER_MM], _F32, tag="ps", name="pstrip"
                )
                for dc in range(KW):
                    nc.tensor.matmul(
                        pstrip[:STRIP_ROWS, :STRIP_COLS],
                        wt[:STRIP_IN, dc * PANEL_OUT : dc * PANEL_OUT + STRIP_ROWS],
                        xst[:STRIP_IN, dc : dc + STRIP_COLS],
                        start=(dc == 0),
                        stop=(dc == KW - 1),
                        skip_group_check=True,
                    )
                ost = cpool.tile([128, STRIP_COLS], _BF16, name="ost")
                nc.vector.tensor_scalar_add(
                    ost[:STRIP_ROWS, :],
                    pstrip[:STRIP_ROWS, :STRIP_COLS],
                    bt[:STRIP_ROWS, :],
                )
                # SP ring: the load queue is idle by now, so the final
                # small store doesn't wait behind the big panel stores.
                nc.sync.dma_start(ys[:, :], ost[:STRIP_ROWS, :])

            # --- 4 full-width panels; the strip compute is emitted LAST
            # so the kernel's final store is the tiny 63 KB ys while the
            # last big panel store drains in parallel. Its input load is
            # dispatched after the panel-0 sections (it has ~20us slack).
            for panel in range(N_PANELS):
                if panel == 1:
                    nc.sync.dma_start(xst[:STRIP_IN, :], xs[:, :])
                r0 = PANEL_OUT * panel

                # (tile_col0, load width, groups fed by this load)
                if panel == 0:
                    # Small leading sub-loads so the first panel matmuls
                    # start as early as possible after the DMA preamble.
                    sections = [
                        (0, 514, (0,)),
                        (512, 1026, (1, 2)),
                        (1536, 1026, (3, 4)),
                        (2560, W - 2560, (5, 6, 7)),
                    ]
                else:
                    sections = [(0, W, tuple(range(N_GROUPS)))]

                ot = opool.tile([128, OW], _BF16)
                xts = {}
                for t0c, cw, groups in sections:
                    xt = xpool.tile(
                        [128, cw], _BF16, tag=f"x{cw}",
                        bufs=3 if cw == W else 2, name=f"x{cw}",
                    )
                    nc.sync.dma_start(
                        xt[:128, :cw], x[r0 : r0 + 128, t0c : t0c + cw]
                    )
                    for jj in groups:
                        xts[jj] = (xt, t0c)

                # Two matmul chunks per panel so the first half's drains +
                # store overlap the second half's matmuls -- including in
                # the LAST panel, where this halves the un-overlapped
                # store tail. Panel 0's halves align with its load
                # sections so no matmul waits on a later section.
                if panel == 0:
                    halves = (
                        (0, (0, 1, 2), 0, 1536),
                        (1, (3, 4, 5, 6, 7), 1536, OW - 1536),
                    )
                else:
                    halves = (
                        (0, (0, 1, 2, 3), 0, CHUNK),
                        (1, (4, 5, 6, 7), CHUNK, OW - CHUNK),
                    )
                for half, groups, g0s, sw in halves:
                    pss = {
                        jj: ppool.tile(
                            [128, COLS_PER_MM], _F32, tag="ps", name=f"ps{jj}"
                        )
                        for jj in groups
                    }
                    for dc in range(KW):
                        for jj in groups:
                            c0 = jj * COLS_PER_MM
                            N = min(COLS_PER_MM, OW - c0)  # 512 / 510
                            xt, t0c = xts[jj]
                            nc.tensor.matmul(
                                pss[jj][:PANEL_OUT, :N],
                                wt[:128, dc * PANEL_OUT : dc * PANEL_OUT + PANEL_OUT],
                                xt[:128, c0 - t0c + dc : c0 - t0c + dc + N],
                                start=(dc == 0),
                                stop=(dc == KW - 1),
                                skip_group_check=True,
                            )
                    for jj in groups:
                        c0 = jj * COLS_PER_MM
                        N = min(COLS_PER_MM, OW - c0)
                        if jj % 2 == 0:
                            nc.scalar.activation(
                                ot[:PANEL_OUT, c0 : c0 + N],
                                pss[jj][:PANEL_OUT, :N],
                                mybir.ActivationFunctionType.Identity,
                                bias=bt[:PANEL_OUT, :],
                            )
                        else:
                            nc.vector.tensor_scalar_add(
                                ot[:PANEL_OUT, c0 : c0 + N],
                                pss[jj][:PANEL_OUT, :N],
                                bt[:PANEL_OUT, :],
                            )
                    # Store each half as soon as its drains land. Late
                    # second-half stores ride the (idle-by-then) SP ring
                    # so the two rings drain the tail in parallel.
                    ring = nc.sync if (half == 1 and panel >= 2) else nc.scalar
                    ring.dma_start(
                        y[r0 : r0 + PANEL_OUT, g0s : g0s + sw],
                        ot[:PANEL_OUT, g0s : g0s + sw],
                    )
            emit_strip()
    nc.compile()
    return nc


def _banded_weights(weight: np.ndarray) -> np.ndarray:
    """lhsT for each kernel column dc, laid out as [128, KW*PANEL_OUT].

    wT[k, dc*PANEL_OUT + m] = weight[k - m, dc] for 0 <= k - m < KH.
    The strip's [STRIP_IN, STRIP_ROWS] banded matrix is the top-left
    block of the same layout, so one tensor serves both shapes.
    """
    wT = np.zeros((128, KW * PANEL_OUT + 2), np.float32)
    m = np.arange(PANEL_OUT)
    for dc in range(KW):
        for d in range(KH):
            wT[m + d, dc * PANEL_OUT + m] = weight[d, dc]
    return wT.astype(_NP_BF16)


def _install_ntff_hook():
    """Shim antenv.axon_hooks so run_bass_kernel_spmd(trace=True) can find
    the axon NTFF profiling hook (the image's antenv lacks axon_hooks)."""
    import sys
    import types

    try:
        from antenv.axon_hooks import get_axon_ntff_profile_hook  # noqa: F401

        return
    except ImportError:
        pass
    import antenv
    from trn_agent_boot.trn_boot import _ntff_profile_via_ctypes

    hook = _ntff_profile_via_ctypes("/opt/axon/libaxon_pjrt.so")
    mod = types.ModuleType("antenv.axon_hooks")
    mod._hook = hook
    mod.set_axon_ntff_profile_hook = lambda h: setattr(mod, "_hook", h)
    mod.get_axon_ntff_profile_hook = lambda: mod._hook
    sys.modules["antenv.axon_hooks"] = mod
    antenv.axon_hooks = mod


def kernel(x, weight, bias, _trace=False, _trace_cores=None):
    global _PROGRAM_CACHE, last_results
    if _trace:
        _install_ntff_hook()
    x = np.asarray(x, dtype=np.float32).astype(_NP_BF16)
    weight = np.asarray(weight, dtype=np.float32)
    bias = np.asarray(bias, dtype=np.float32)

    if _PROGRAM_CACHE is None:
        _PROGRAM_CACHE = _build_program()
    nc = _PROGRAM_CACHE

    wT = _banded_weights(weight)
    wT[:, KW * PANEL_OUT] = _NP_BF16(bias[0])

    # Strip input: rows STRIP_R0..H, columns sharded across cores with a
    # 2-col halo; the last core's tail is zero-padded (its last 2 strip
    # output cols are garbage and discarded below).
    xpad = np.zeros((STRIP_IN, NCORES * STRIP_COLS + STRIP_IN_COLS - STRIP_COLS),
                    _NP_BF16)
    xpad[:, :W] = x[STRIP_R0:, :]

    in_maps = []
    for i in range(NCORES):
        r0 = i * ROWS_PER_CORE
        in_maps.append(
            {
                "x": np.ascontiguousarray(x[r0 : r0 + IN_ROWS]),
                "xs": np.ascontiguousarray(
                    xpad[:, i * STRIP_COLS : i * STRIP_COLS + STRIP_IN_COLS]
                ),
                "wb": wT,
            }
        )

    kwargs = {}
    if _trace:
        kwargs["trace"] = True
        kwargs["trace_cores"] = (
            list(range(NCORES)) if _trace_cores is None else _trace_cores
        )
    res = run_bass_kernel_spmd(nc, in_maps, core_ids=list(range(NCORES)), **kwargs)
    last_results = res

    out = np.empty((OH, OW), np.float32)
    for i in range(NCORES):
        out[i * ROWS_PER_CORE : (i + 1) * ROWS_PER_CORE] = res.results[i][
            "y"
        ].astype(np.float32)
        c0 = i * STRIP_COLS
        cw = min(STRIP_COLS, OW - c0)
        out[STRIP_R0:, c0 : c0 + cw] = res.results[i]["ys"][:, :cw].astype(
            np.float32
        )
    return out
